# revision 2
# baseline (speedup 1.0000x reference)
"""Trainium2 Bass kernel: multi-adapter LoRA linear via host-side fold,
computed entirely in fp8e4m3 DoubleRow matmuls with residual correction.

y = x @ W.T + bias + 2*(x@A_g.T)@B_g.T  ==  x @ (W + 2*B_g@A_g).T + bias

Data-parallel over batch: each of 8 cores gets one batch element and its
group's folded weight matrix W'_g = W + 2*B_g@A_g. The matmul runs on the
PE in fp8e4m3 DoubleRow mode (2 k-chunks per instruction at 0.5 cycles per
output row = 4x fp16 throughput). Precision is recovered with residual
operands, all at inverse-paired scales (x*XS, W*WS with XS*WS=1) so every
product lands in PSUM at the true scale and accumulates directly:

  per corrected chunk pair (j,j+1), 3 DoubleRow instructions:
    DR_A: x8 @ w8        main product        (x8 = q8(x*XS), w8 = q8(W'*WS))
    DR_B: xr8 @ w8       x-residual fix      (xr8 = q8(x*XS - x8))
    DR_C: x8 @ wlo       W-residual fix      (wlo = q8(W'*WS - w8))
  per pure chunk pair, 1 DoubleRow instruction (DR_A only).

NFC=24 corrected + NP=8 pure chunks gives rel_rms ~1.6e-2 (< 2e-2 gate)
at 10240 PE cycles per output tile vs 16384 for fp16 (-37.5%).

Schedule: startup stream rides one queue in deadline order (per corrected
pair: x8, w8(ob0), xr8, wlo(ob0); bias; then pure pairs); phase A
interleaves each pair's 3 matmuls across the first 8 output tiles (8 PSUM
banks) so the PE saturates as data arrives. Remaining obs prefetch W one
tile ahead; out tiles leave via DVE bias-add + DMA on the other queue.
"""
import sys

if "/opt/trn_rl_repo" not in sys.path:
    sys.path.insert(0, "/opt/trn_rl_repo")

import numpy as np
import ml_dtypes

B, S, I, O, G, R = 8, 2048, 4096, 4096, 4, 16
OB = 512          # output free-dim tile (one PSUM bank of fp32)
NFC = 24          # residual-corrected fp8 k-chunks (must be even)
NP = 8            # pure fp8 k-chunks (must be even, NFC+NP = I/128)
XS, WS = 0.125, 8.0   # inverse-paired operand scales (XS*WS == 1)
F8 = ml_dtypes.float8_e4m3

_CACHE = {}


def build(s=S, i=I, o=O, repeat=1):
    import concourse.bacc as bacc
    import concourse.mybir as mybir
    import concourse.tile as tile

    f8, f16, f32 = mybir.dt.float8e4, mybir.dt.float16, mybir.dt.float32
    DR = mybir.MatmulPerfMode.DoubleRow
    kt = i // 128
    ktc = NFC
    mt_n = s // 128
    nob = o // OB

    nc = bacc.Bacc("TRN2", target_bir_lowering=False, debug=False)
    x8d = nc.dram_tensor("x8", [i, s], f8, kind="ExternalInput").ap()
    xr8d = nc.dram_tensor("xr8", [ktc * 128, s], f8, kind="ExternalInput").ap()
    w8d = nc.dram_tensor("w8", [i, o], f8, kind="ExternalInput").ap()
    wlod = nc.dram_tensor("wlo", [ktc * 128, o], f8, kind="ExternalInput").ap()
    biasr = nc.dram_tensor("biasr", [128, o], f16, kind="ExternalInput").ap()
    out = nc.dram_tensor("out", [s, o], f32, kind="ExternalOutput").ap()

    with tile.TileContext(nc) as tc:
        with (
            tc.tile_pool(name="xp", bufs=1) as xp,
            tc.tile_pool(name="wp", bufs=2) as wp,
            tc.tile_pool(name="lp", bufs=2) as lp,
            tc.tile_pool(name="bp", bufs=1) as bp,
            tc.tile_pool(name="op", bufs=2) as op,
            tc.tile_pool(name="pp", bufs=8, space="PSUM") as pp,
        ):
          x8t3 = x8d.rearrange("(k p) s -> p k s", p=128)    # [128, kt, s]
          xr8t3 = xr8d.rearrange("(k p) s -> p k s", p=128)  # [128, ktc, s]
          w8t3 = w8d.rearrange("(k p) o -> p k o", p=128)    # [128, kt, o]
          wlot3 = wlod.rearrange("(k p) o -> p k o", p=128)  # [128, ktc, o]

          for _rep in range(repeat):
            x8_sb = xp.tile([128, kt, s], f8, name="x8_sb")
            xr8_sb = xp.tile([128, ktc, s], f8, name="xr8_sb")
            bias_sb = bp.tile([128, o], f16)

            w8_0 = wp.tile([128, kt, OB], f8, tag="w8")
            wlo_0 = lp.tile([128, ktc, OB], f8, tag="wlo")
            # Startup stream in strict deadline order on one queue: phase A
            # consumes pair j's DR_A (x8+w8), DR_B (xr8), DR_C (wlo) in that
            # order, so deliver per-pair in that order. Bias before the pure
            # pairs (needed at first evac, just after the last pure pair).
            for j in range(0, ktc, 2):
                nc.sync.dma_start(x8_sb[:, j:j + 2, :], x8t3[:, j:j + 2, :])
                nc.sync.dma_start(w8_0[:, j:j + 2, :], w8t3[:, j:j + 2, 0:OB])
                nc.sync.dma_start(xr8_sb[:, j:j + 2, :], xr8t3[:, j:j + 2, :])
                nc.sync.dma_start(wlo_0[:, j:j + 2, :], wlot3[:, j:j + 2, 0:OB])
            nc.sync.dma_start(bias_sb[:, :], biasr[:, :])
            for j in range(ktc, kt, 2):
                nc.sync.dma_start(x8_sb[:, j:j + 2, :], x8t3[:, j:j + 2, :])
                nc.sync.dma_start(w8_0[:, j:j + 2, :], w8t3[:, j:j + 2, 0:OB])

            def dr(pt, stat_sb, mov_sb, j, mt, start, stop):
                nc.tensor.matmul(
                    pt[:],
                    stat_sb[:, j:j + 2, mt * 128:mt * 128 + 128],
                    mov_sb[:, j:j + 2, :],
                    start=start,
                    stop=stop,
                    perf_mode=DR,
                )

            def tile_mms(pt, w8t, wlot, mt):
                for j in range(0, ktc, 2):
                    dr(pt, x8_sb, w8t, j, mt, j == 0, False)
                    dr(pt, xr8_sb, w8t, j, mt, False, False)
                    dr(pt, x8_sb, wlot, j, mt, False, False)
                for j in range(ktc, kt, 2):
                    dr(pt, x8_sb, w8t, j, mt, False, j == kt - 2)

            def evac(pt, mt, ob):
                ot = op.tile([128, OB], f32, tag="ot")
                nc.vector.tensor_tensor(
                    ot[:], pt[:], bias_sb[:, ob * OB:(ob + 1) * OB],
                    op=mybir.AluOpType.add,
                )
                nc.scalar.dma_start(
                    out[mt * 128:(mt + 1) * 128, ob * OB:(ob + 1) * OB], ot[:]
                )

            def load_w(ob):
                w = wp.tile([128, kt, OB], f8, tag="w8")
                wl = lp.tile([128, ktc, OB], f8, tag="wlo")
                nc.sync.dma_start(w[:, :, :], w8t3[:, :, ob * OB:(ob + 1) * OB])
                nc.sync.dma_start(wl[:, :, :], wlot3[:, :, ob * OB:(ob + 1) * OB])
                return w, wl

            # phase A: per chunk pair, the pair's matmuls across the first 8
            # output tiles -- each arriving pair enables 8-24 matmuls, the PE
            # saturates as the startup stream lands
            nA = min(8, mt_n)
            pts = [pp.tile([128, OB], f32, tag="pt", name=f"ptA{m}") for m in range(nA)]
            for j in range(0, ktc, 2):
                for m in range(nA):
                    dr(pts[m], x8_sb, w8_0, j, m, j == 0, False)
                for m in range(nA):
                    dr(pts[m], xr8_sb, w8_0, j, m, False, False)
                for m in range(nA):
                    dr(pts[m], x8_sb, wlo_0, j, m, False, False)
            for j in range(ktc, kt, 2):
                for m in range(nA):
                    dr(pts[m], x8_sb, w8_0, j, m, False, j == kt - 2)
            w_cur = load_w(1) if nob > 1 else None
            for m in range(nA):
                evac(pts[m], m, 0)

            # rest of ob0 (all operands resident by now)
            for mt in range(nA, mt_n):
                pt = pp.tile([128, OB], f32, tag="pt")
                tile_mms(pt, w8_0, wlo_0, mt)
                evac(pt, mt, 0)

            # remaining obs, W prefetched one ahead on the sync queue
            for ob in range(1, nob):
                w, wl = w_cur
                w_cur = load_w(ob + 1) if ob + 1 < nob else None
                for mt in range(mt_n):
                    pt = pp.tile([128, OB], f32, tag="pt")
                    tile_mms(pt, w, wl, mt)
                    evac(pt, mt, ob)
    nc.compile()
    return nc


def prep_in_maps(data, W, bias, lora_a, lora_b):
    biasr = np.ascontiguousarray(
        np.broadcast_to(bias.astype(np.float16), (128, W.shape[0]))
    )
    W8g, Wlog = {}, {}
    for g in range(G):
        Wg = W.astype(np.float32) + 2.0 * (
            lora_b[g].astype(np.float32) @ lora_a[g].astype(np.float32)
        )
        Ws = np.ascontiguousarray(Wg.T * WS)          # [I, O] scaled
        w8 = Ws.astype(F8)
        wlo = (Ws[:NFC * 128] - w8[:NFC * 128].astype(np.float32)).astype(F8)
        W8g[g], Wlog[g] = w8, wlo
    in_maps = []
    for b in range(data.shape[0]):
        g = b // (data.shape[0] // G)
        xs = np.ascontiguousarray(data[b].T * np.float32(XS))  # [I, S] scaled
        x8 = xs.astype(F8)
        xr8 = (xs[:NFC * 128] - x8[:NFC * 128].astype(np.float32)).astype(F8)
        in_maps.append({
            "x8": x8,
            "xr8": xr8,
            "w8": W8g[g],
            "wlo": Wlog[g],
            "biasr": biasr,
        })
    return in_maps


def kernel(data, W, bias, lora_a, lora_b):
    from concourse.bass_utils import run_bass_kernel_spmd

    if "nc" not in _CACHE:
        _CACHE["nc"] = build()
    nc = _CACHE["nc"]
    in_maps = prep_in_maps(data, W, bias, lora_a, lora_b)
    res = run_bass_kernel_spmd(nc, in_maps, list(range(len(in_maps))))
    return np.stack([res.results[c]["out"] for c in range(len(in_maps))], axis=0)


# revision 3
# speedup vs baseline: 1.4693x; 1.4693x over previous
"""Trainium2 Bass kernel: multi-adapter LoRA linear via host-side fold,
fp16 matmul with an fp8e4m3 DoubleRow hybrid contraction.

y = x @ W.T + bias + 2*(x@A_g.T)@B_g.T  ==  x @ (W + 2*B_g@A_g).T + bias

Data-parallel over batch: each of 8 cores gets one batch element and its
group's folded weight matrix W'_g = W + 2*B_g@A_g. The contraction dim
(32 chunks of 128) is split: the first 22 chunks run in fp16 (1 cycle per
output row each), the last 10 run in fp8e4m3 DoubleRow mode (2 chunks per
instruction at 1 cycle per output row -- measured: DoubleRow is 2x fp16
per chunk on this hw, not the cost model's 4x). 13824 PE cycles per
output tile vs 16384 all-fp16 (-15.6%). fp8 operands use inverse-paired
scales (x*XS, W*WS with XS*WS == 1) so fp8 products accumulate into PSUM
at the true scale alongside the fp16 products; rel_rms ~1.8e-2 vs the
2e-2 gate (deterministic inputs).

Schedule: the startup stream (x + W'(ob0) chunks, interleaved per-k in
deadline order, bias, then the fp8 pairs) rides one queue since all DMA
shares one engine; phase A interleaves each chunk's matmuls across the
first 8 output tiles (8 PSUM banks) so the PE saturates ~2us in.
Remaining obs prefetch W one tile ahead; out tiles leave via DVE
bias-add + DMA on the other queue.
"""
import sys

if "/opt/trn_rl_repo" not in sys.path:
    sys.path.insert(0, "/opt/trn_rl_repo")

import numpy as np
import ml_dtypes

B, S, I, O, G, R = 8, 2048, 4096, 4096, 4, 16
OB = 512          # output free-dim tile (one PSUM bank of fp32)
N8 = 10           # fp8 k-chunks (must be even; the last N8 of 32)
K16 = I // 128 - N8   # fp16 k-chunks
XS, WS = 0.125, 8.0   # inverse-paired fp8 operand scales (XS*WS == 1)
F8 = ml_dtypes.float8_e4m3

_CACHE = {}


def build(s=S, i=I, o=O, repeat=1):
    import concourse.bacc as bacc
    import concourse.mybir as mybir
    import concourse.tile as tile

    f8, f16, f32 = mybir.dt.float8e4, mybir.dt.float16, mybir.dt.float32
    DR = mybir.MatmulPerfMode.DoubleRow
    kt = i // 128
    k16 = K16
    mt_n = s // 128
    nob = o // OB

    nc = bacc.Bacc("TRN2", target_bir_lowering=False, debug=False)
    x16d = nc.dram_tensor("x16", [k16 * 128, s], f16, kind="ExternalInput").ap()
    x8d = nc.dram_tensor("x8", [N8 * 128, s], f8, kind="ExternalInput").ap()
    w16d = nc.dram_tensor("w16", [k16 * 128, o], f16, kind="ExternalInput").ap()
    w8d = nc.dram_tensor("w8", [N8 * 128, o], f8, kind="ExternalInput").ap()
    biasr = nc.dram_tensor("biasr", [128, o], f16, kind="ExternalInput").ap()
    out = nc.dram_tensor("out", [s, o], f32, kind="ExternalOutput").ap()

    with tile.TileContext(nc) as tc:
        with (
            tc.tile_pool(name="xp", bufs=1) as xp,
            tc.tile_pool(name="wp", bufs=2) as wp,
            tc.tile_pool(name="lp", bufs=2) as lp,
            tc.tile_pool(name="bp", bufs=1) as bp,
            tc.tile_pool(name="op", bufs=2) as op,
            tc.tile_pool(name="pp", bufs=8, space="PSUM") as pp,
        ):
          x16t3 = x16d.rearrange("(k p) s -> p k s", p=128)  # [128, k16, s]
          x8t3 = x8d.rearrange("(k p) s -> p k s", p=128)    # [128, N8, s]
          w16t3 = w16d.rearrange("(k p) o -> p k o", p=128)  # [128, k16, o]
          w8t3 = w8d.rearrange("(k p) o -> p k o", p=128)    # [128, N8, o]

          for _rep in range(repeat):
            x16_sb = xp.tile([128, k16, s], f16, name="x16_sb")
            x8_sb = xp.tile([128, N8, s], f8, name="x8_sb")
            bias_sb = bp.tile([128, o], f16)

            w16_0 = wp.tile([128, k16, OB], f16, tag="w16")
            w8_0 = lp.tile([128, N8, OB], f8, tag="w8")
            # Startup stream in strict deadline order on one queue: phase A
            # consumes fp16 chunk k and W0 chunk k at ~1.7us/k while DMA
            # delivers the pair in ~1.8us; interleave per-k so the deficit
            # stays minimal. Bias before the fp8 pairs (first evac happens
            # right after the last fp8 pair).
            nc.sync.dma_start(x16_sb[:, 0:1, :], x16t3[:, 0:1, :])
            nc.sync.dma_start(w16_0[:, 0:1, :], w16t3[:, 0:1, 0:OB])
            nc.sync.dma_start(x16_sb[:, 1:2, :], x16t3[:, 1:2, :])
            nc.sync.dma_start(w16_0[:, 1:2, :], w16t3[:, 1:2, 0:OB])
            for k in range(2, k16, 2):
                nc.sync.dma_start(x16_sb[:, k:k + 2, :], x16t3[:, k:k + 2, :])
                nc.sync.dma_start(w16_0[:, k:k + 2, :], w16t3[:, k:k + 2, 0:OB])
            nc.sync.dma_start(bias_sb[:, :], biasr[:, :])
            for j in range(0, N8, 2):
                nc.sync.dma_start(x8_sb[:, j:j + 2, :], x8t3[:, j:j + 2, :])
                nc.sync.dma_start(w8_0[:, j:j + 2, :], w8t3[:, j:j + 2, 0:OB])

            def mm16(pt, w, mt, k, start):
                nc.tensor.matmul(
                    pt[:],
                    x16_sb[:, k, mt * 128:mt * 128 + 128],
                    w[:, k, :],
                    start=start,
                    stop=False,
                )

            def mm8(pt, w8, mt, j, stop):
                nc.tensor.matmul(
                    pt[:],
                    x8_sb[:, j:j + 2, mt * 128:mt * 128 + 128],
                    w8[:, j:j + 2, :],
                    start=False,
                    stop=stop,
                    perf_mode=DR,
                )

            def tile_mms(pt, w, w8, mt):
                for k in range(k16):
                    mm16(pt, w, mt, k, k == 0)
                for j in range(0, N8, 2):
                    mm8(pt, w8, mt, j, j == N8 - 2)

            def evac(pt, mt, ob):
                ot = op.tile([128, OB], f32, tag="ot")
                nc.vector.tensor_tensor(
                    ot[:], pt[:], bias_sb[:, ob * OB:(ob + 1) * OB],
                    op=mybir.AluOpType.add,
                )
                nc.scalar.dma_start(
                    out[mt * 128:(mt + 1) * 128, ob * OB:(ob + 1) * OB], ot[:]
                )

            def load_w(ob):
                w = wp.tile([128, k16, OB], f16, tag="w16")
                w8 = lp.tile([128, N8, OB], f8, tag="w8")
                nc.sync.dma_start(w[:, :, :], w16t3[:, :, ob * OB:(ob + 1) * OB])
                nc.sync.dma_start(w8[:, :, :], w8t3[:, :, ob * OB:(ob + 1) * OB])
                return w, w8

            # phase A: per k-chunk, matmuls of the first nA tiles of ob0 --
            # each arriving chunk enables nA matmuls, PE saturates early
            nA = min(8, mt_n)
            pts = [pp.tile([128, OB], f32, tag="pt", name=f"ptA{m}") for m in range(nA)]
            for k in range(k16):
                for m in range(nA):
                    mm16(pts[m], w16_0, m, k, k == 0)
            for j in range(0, N8, 2):
                for m in range(nA):
                    mm8(pts[m], w8_0, m, j, j == N8 - 2)
            w_cur = load_w(1) if nob > 1 else None
            for m in range(nA):
                evac(pts[m], m, 0)

            # rest of ob0 (all operands resident by now)
            for mt in range(nA, mt_n):
                pt = pp.tile([128, OB], f32, tag="pt")
                tile_mms(pt, w16_0, w8_0, mt)
                evac(pt, mt, 0)

            # remaining obs, W prefetched one ahead on the sync queue
            for ob in range(1, nob):
                w, w8 = w_cur
                w_cur = load_w(ob + 1) if ob + 1 < nob else None
                for mt in range(mt_n):
                    pt = pp.tile([128, OB], f32, tag="pt")
                    tile_mms(pt, w, w8, mt)
                    evac(pt, mt, ob)
    nc.compile()
    return nc


def prep_in_maps(data, W, bias, lora_a, lora_b):
    k16r = K16 * 128
    biasr = np.ascontiguousarray(
        np.broadcast_to(bias.astype(np.float16), (128, W.shape[0]))
    )
    W16g, W8g = {}, {}
    for g in range(G):
        Wg = W.astype(np.float32) + 2.0 * (
            lora_b[g].astype(np.float32) @ lora_a[g].astype(np.float32)
        )
        WT = Wg.T  # [I, O]
        W16g[g] = np.ascontiguousarray(WT[:k16r]).astype(np.float16)
        W8g[g] = np.ascontiguousarray(WT[k16r:] * np.float32(WS)).astype(F8)
    in_maps = []
    for b in range(data.shape[0]):
        g = b // (data.shape[0] // G)
        xT = data[b].T  # [I, S]
        in_maps.append({
            "x16": np.ascontiguousarray(xT[:k16r]).astype(np.float16),
            "x8": np.ascontiguousarray(xT[k16r:] * np.float32(XS)).astype(F8),
            "w16": W16g[g],
            "w8": W8g[g],
            "biasr": biasr,
        })
    return in_maps


def kernel(data, W, bias, lora_a, lora_b):
    from concourse.bass_utils import run_bass_kernel_spmd

    if "nc" not in _CACHE:
        _CACHE["nc"] = build()
    nc = _CACHE["nc"]
    in_maps = prep_in_maps(data, W, bias, lora_a, lora_b)
    res = run_bass_kernel_spmd(nc, in_maps, list(range(len(in_maps))))
    return np.stack([res.results[c]["out"] for c in range(len(in_maps))], axis=0)
